# revision 53
# baseline (speedup 1.0000x reference)
"""GAT layer on 8 Trainium2 NeuronCores (Bass/Tile).

Strategy (edge partition by dst, standard 1D graph parallelism):
- Each core owns a contiguous slice of 6250 dst nodes (all edges into them).
- Nodes within a core are re-ordered by in-degree (host-side index prep) so
  that each 128-node PSUM window has near-uniform degree; edge slot (p, c)
  holds the c-th incoming edge of window-node p.  The segment scatter-sum
  then degenerates to matmul-accumulate with a constant identity stationary
  operand, and er[dst] broadcasts per-partition (no per-edge gather).
- Projection h = feat @ [W | Wl | Wr] is node-parallel; the per-node table
  row (768 B: h bf16 + el f32) is AllGathered, then rows are fetched
  per-edge with dma_gather (SWDGE, 4 queues, multi-packet; pad slots rotate
  over 88 pad rows to dodge HBM same-bank serialization).  Softmax runs
  unnormalized (logits are O(1) for this operator) and normalization
  happens per node after aggregation.
- Node->core assignment balances every dst node's in-edges across table
  halves A/B (greedy discrepancy minimization) so the per-window A and B
  slot grids each need only ~deg/2 columns.
"""
import math
import numpy as np
import sys

sys.path.insert(0, "/opt/trn_rl_repo")

from concourse import bass, mybir, bacc, tile
from concourse.bass_utils import run_bass_kernel_spmd

N_NODES = 50000
N_EDGES = 800000
IN_FEATS = 256
NUM_HEADS = 8
OUT_FEATS = 32
HF = NUM_HEADS * OUT_FEATS          # 256
NEG_SLOPE = 0.2
N_CORES = 8
NPC = N_NODES // N_CORES            # 6250 nodes per core
NPAD = 6272                         # 49 * 128
NWIN = NPAD // 128                  # 49
ROWT = 256                          # table row u16 lanes (512 B):
                                    #   lane 0-3:  h[0:8] fp8
                                    #   lane 4-7:  el[0:8] fp8
                                    #   lane 8-255: h[8:256] bf16
TROWS = N_CORES * NPAD              # 50176
HALF = TROWS // 2                   # 25088
PAD_REL = 6271                      # pad row, relative index in either half
NREAL = 6250                        # real nodes per core (rest are pad rows)
PAD_EL = -448.0                     # pad rows' el (fp8 min) -> exp() == 0
MAXC = 16                           # chunk columns per dma_gather
NSQ = 4                             # SWDGE queues
F32 = mybir.dt.float32
BF16 = mybir.dt.bfloat16
U16 = mybir.dt.uint16
I16 = mybir.dt.int16
FP8 = mybir.dt.float8e4


def _degree_rank(degA_slice, degB_slice):
    """Rank nodes by (degA desc, degB desc); with per-dst A/B edge balancing
    degA ~= degB, so windows sorted by degA are near-homogeneous in both
    halves and padding ~= spread of total degree only."""
    order = np.lexsort((-degB_slice, -degA_slice))
    rank = np.empty_like(order)
    rank[order] = np.arange(len(order))
    return order, rank


def _balance_halves(src, dst, deg):
    """Assign nodes to cores so that (a) each degree-octet spreads across all
    8 cores (stratification: per-core degree profiles match) and (b) every
    dst node's in-edges split evenly between src-half A (cores 0-3) and B
    (cores 4-7).  (b) minimizes slot-grid padding: the A and B grids for a
    window each need only ~deg/2 columns."""
    gorder = np.argsort(-deg, kind="stable")
    order_by_src = np.argsort(src, kind="stable")
    dst_sorted = dst[order_by_src]
    starts = np.searchsorted(src[order_by_src], np.arange(N_NODES + 1))
    imb = np.zeros(N_NODES, np.int32)          # degA - degB per dst node
    core_of = np.empty(N_NODES, np.int64)
    for oi in range(0, N_NODES, 8):
        octet = gorder[oi:oi + 8]
        capA = capB = 4
        iA = iB = 0
        for v in octet:
            nb = dst_sorted[starts[v]:starts[v + 1]]
            if capA == 0:
                toA = False
            elif capB == 0:
                toA = True
            elif len(nb) == 0:
                toA = capA >= capB
            else:
                cur = imb[nb]
                cA = np.abs(cur + 1).sum()
                cB = np.abs(cur - 1).sum()
                toA = cA <= cB
            if toA:
                core_of[v] = iA
                iA += 1
                capA -= 1
                if len(nb):
                    imb[nb] += 1
            else:
                core_of[v] = 4 + iB
                iB += 1
                capB -= 1
                if len(nb):
                    imb[nb] -= 1
    return core_of


def _grid_for(p_arr, rel_arr):
    """Edges (partition p, relative table idx) -> grid [128, ncx] int32,
    pad slots = PAD_REL.  Each partition's edges sorted ascending by rel."""
    if len(p_arr) == 0:
        return np.full((128, 1), PAD_REL, np.int32)
    o = np.lexsort((rel_arr, p_arr))
    p_arr, rel_arr = p_arr[o], rel_arr[o]
    counts = np.bincount(p_arr, minlength=128)
    ncx = int(counts.max())
    grid = np.full((128, ncx), PAD_REL, np.int32)
    # column index within each partition = running position
    starts = np.zeros(128, np.int64)
    starts[1:] = np.cumsum(counts)[:-1]
    col = np.arange(len(p_arr)) - starts[p_arr]
    grid[p_arr, col] = rel_arr
    return grid


def _pack_gather(grid_cols):
    """grid_cols [128, cols] -> wrapped idx image [128, n/16] int16."""
    cols = grid_cols.shape[1]
    n = 128 * cols
    flat = np.empty(n, np.int32)
    P = np.arange(128)[:, None]
    CI = np.arange(cols)[None, :]
    pos = (P % 16) * (n // 16) + (P // 16) + 8 * CI
    flat[pos.ravel()] = grid_cols.ravel()
    return np.tile(flat.reshape(16, n // 16), (8, 1)).astype(np.int16)


def _prep(src, dst):
    """All host-side index preprocessing.  Returns per-core arrays + the
    (shared) gather schedule."""
    # Stratified node->core assignment with per-dst A/B edge balancing.
    deg = np.bincount(dst, minlength=N_NODES)
    core_of = _balance_halves(src, dst, deg)
    srcA = core_of[src] < (N_CORES // 2)        # src in table half A
    degA = np.bincount(dst[srcA], minlength=N_NODES)
    degB = np.bincount(dst[~srcA], minlength=N_NODES)
    grank = np.empty(N_NODES, np.int64)
    perms = []
    for c in range(N_CORES):
        ids = np.nonzero(core_of == c)[0]       # global node ids of core c
        order, rank = _degree_rank(degA[ids], degB[ids])
        perms.append(ids[order])                # rank -> global node id
        grank[ids] = c * NPAD + rank
    sg_all = grank[src]
    dcore = core_of[dst]
    dr_all = grank[dst] - dcore * NPAD          # local rank of dst

    # per (core, window, half) grids
    grids = [[[None, None] for _ in range(NWIN)] for _ in range(N_CORES)]
    ncx = np.zeros((N_CORES, NWIN, 2), np.int64)
    for c in range(N_CORES):
        sel = np.nonzero(dcore == c)[0]
        sg = sg_all[sel]
        dr = dr_all[sel]
        w_arr = dr // 128
        p_arr = dr % 128
        half = (sg >= HALF).astype(np.int64)
        rel = np.where(half == 0, sg, sg - HALF)
        key = w_arr * 2 + half
        order = np.argsort(key, kind="stable")
        ksort = key[order]
        bounds = np.searchsorted(ksort, np.arange(NWIN * 2 + 1))
        for w in range(NWIN):
            for h in range(2):
                lo, hi = bounds[w * 2 + h], bounds[w * 2 + h + 1]
                idxs = order[lo:hi]
                g = _grid_for(p_arr[idxs].astype(np.int64),
                              rel[idxs].astype(np.int64))
                grids[c][w][h] = g
                ncx[c, w, h] = g.shape[1]

    # shared schedule: per (window, half) chunk count = max over cores
    ncA = ncx[:, :, 0].max(axis=0)
    ncB = ncx[:, :, 1].max(axis=0)
    ncA = np.maximum(ncA, 1)                    # ensure >=1 chunk per window
    sched = []                                  # (w, half, cols, off16)
    off16 = 0
    for w in range(NWIN):
        for h, nc_w in ((0, int(ncA[w])), (1, int(ncB[w]))):
            for j0 in range(0, nc_w, MAXC):
                cols = min(MAXC, nc_w - j0)
                if cols <= 0:
                    continue
                sched.append((w, h, cols, off16))
                off16 += cols * 8               # n/16 = 128*cols/16
    toti16 = off16
    # queue = emission index % NSQ: Tile assigns SWDGE DMAs to the 8 DMASW
    # sem lanes round-robin in emission order, and each lane must stay
    # locked to a single SWDGE queue (8 % NSQ == 0 keeps them aligned).
    queues = [i % NSQ for i in range(len(sched))]

    # per-core idx images following the shared schedule.  Pad slots rotate
    # over all 88 pad rows of a half (4 cores x ranks 6250-6271) instead of
    # hammering one row: concurrent same-address HBM reads serialize on a
    # single bank.
    padpool = np.array([c * NPAD + r for c in range(4)
                        for r in range(NREAL, NPAD)], np.int32)
    idx_imgs = []
    for c in range(N_CORES):
        img = np.empty((128, toti16), np.int16)
        phase = 0
        for (w, h, cols, off) in sched:
            g = grids[c][w][h]
            gc = np.full((128, cols), PAD_REL, np.int32)
            # j0 for this gather: count previous gathers of same (w, h)
            prev = sum(cc for (w2, h2, cc, o2) in sched
                       if w2 == w and h2 == h and o2 < off)
            j0 = prev
            avail = max(0, min(cols, g.shape[1] - j0))
            if avail > 0:
                gc[:, :avail] = g[:, j0:j0 + avail]
            mask = gc == PAD_REL
            npads = int(mask.sum())
            if npads:
                gc[mask] = padpool[(np.arange(npads) + phase)
                                   % len(padpool)]
                phase += npads
            img[:, off:off + cols * 8] = _pack_gather(gc)
        idx_imgs.append(img)
    return perms, sg_all, sched, toti16, idx_imgs, ncA, ncB, queues


def _build(sched, toti16, ncA, ncB, queues):
    nc = bacc.Bacc("TRN2", target_bir_lowering=False, debug=False,
                   num_devices=N_CORES, num_swdge_queues=NSQ)
    feat_in = nc.dram_tensor("feat", [NPAD, IN_FEATS], F32,
                             kind="ExternalInput")
    w_in = nc.dram_tensor("w", [IN_FEATS, HF], F32, kind="ExternalInput")
    alb_in = nc.dram_tensor("alb", [128, HF], F32, kind="ExternalInput")
    arb_in = nc.dram_tensor("arb", [128, HF], F32, kind="ExternalInput")
    bias_in = nc.dram_tensor("biasb", [128, HF], F32, kind="ExternalInput")
    id_in = nc.dram_tensor("ident", [128, 128], F32, kind="ExternalInput")
    idb_in = nc.dram_tensor("identb", [128, 128], BF16, kind="ExternalInput")
    idx_in = nc.dram_tensor("idx", [128, toti16], I16, kind="ExternalInput")
    padel_in = nc.dram_tensor("padel", [128, 1], F32, kind="ExternalInput")
    out_d = nc.dram_tensor("out", [NPAD, HF], F32, kind="ExternalOutput")

    with tile.TileContext(nc) as tc:
        with (
            tc.tile_pool(name="const", bufs=1) as constp,
            tc.tile_pool(name="dram", bufs=1, space="DRAM") as dramp,
        ):
            tbl_shard = dramp.tile([NPAD, ROWT], U16)
            er_loc = dramp.tile([NPAD, NUM_HEADS], F32)
            tbl_full = dramp.tile([TROWS, ROWT], U16, addr_space="Shared")

            ident = constp.tile([128, 128], F32)
            nc.sync.dma_start(out=ident[:], in_=id_in[:, :])
            identb = constp.tile([128, 128], BF16)
            nc.sync.dma_start(out=identb[:], in_=idb_in[:, :])
            bias_sb = constp.tile([128, HF], F32)
            nc.sync.dma_start(out=bias_sb[:], in_=bias_in[:, :])
            idx_sb = constp.tile([128, toti16], I16)
            nc.sync.dma_start(out=idx_sb[:], in_=idx_in[:, :])
            alb = constp.tile([128, HF], F32)
            nc.sync.dma_start(out=alb[:], in_=alb_in[:, :])
            arb = constp.tile([128, HF], F32)
            nc.sync.dma_start(out=arb[:], in_=arb_in[:, :])
            padel = constp.tile([128, 1], F32)
            nc.sync.dma_start(out=padel[:], in_=padel_in[:, :])
            c_slope = constp.tile([128, 1], F32)
            nc.vector.memset(c_slope[:], NEG_SLOPE)
            c_eps = constp.tile([128, 1], F32)
            nc.vector.memset(c_eps[:], 1e-30)

            # ---- W_aug: [W | Wl | Wr] per k-half ----
            waug = []
            with tc.tile_pool(name="wtmp", bufs=2) as wtmp:
                for kh in range(2):
                    wa = constp.tile([128, 272], F32, name=f"waug{kh}",
                                     tag=f"waug{kh}")
                    nc.sync.dma_start(out=wa[:, 0:HF],
                                      in_=w_in[kh * 128:(kh + 1) * 128, :])
                    tmp = wtmp.tile([128, HF], F32, tag="wt")
                    nc.vector.tensor_mul(out=tmp[:], in0=wa[:, 0:HF],
                                         in1=alb[:])
                    nc.vector.tensor_reduce(
                        out=wa[:, 256:264],
                        in_=tmp[:].rearrange("p (h f) -> p h f", h=NUM_HEADS),
                        axis=mybir.AxisListType.X, op=mybir.AluOpType.add)
                    tmp2 = wtmp.tile([128, HF], F32, tag="wt2")
                    nc.vector.tensor_mul(out=tmp2[:], in0=wa[:, 0:HF],
                                         in1=arb[:])
                    nc.vector.tensor_reduce(
                        out=wa[:, 264:272],
                        in_=tmp2[:].rearrange("p (h f) -> p h f", h=NUM_HEADS),
                        axis=mybir.AxisListType.X, op=mybir.AluOpType.add)
                    waug.append(wa)

            # ---- projection phase ----
            with (
                tc.tile_pool(name="ft", bufs=6) as ftp,
                tc.tile_pool(name="ftT", bufs=6) as ftTp,
                tc.tile_pool(name="ptr", bufs=4, space="PSUM") as ptrp,
                tc.tile_pool(name="hps", bufs=3, space="PSUM") as hpsp,
                tc.tile_pool(name="hsb", bufs=6) as hsbp,
            ):
                for t in range(NWIN):
                    ft = ftp.tile([128, IN_FEATS], F32, tag="ft")
                    nc.sync.dma_start(
                        out=ft[:], in_=feat_in[t * 128:(t + 1) * 128, :])
                    hp = hpsp.tile([128, 272], F32, tag="hp")
                    ftTs = []
                    for kh in range(2):
                        pt = ptrp.tile([128, 128], F32, tag="pt")
                        nc.tensor.transpose(
                            out=pt[:], in_=ft[:, kh * 128:(kh + 1) * 128],
                            identity=ident[:])
                        ftT = ftTp.tile([128, 128], F32, tag="ftT")
                        nc.vector.tensor_copy(out=ftT[:], in_=pt[:])
                        ftTs.append(ftT)
                    for kh in range(2):
                        nc.tensor.matmul(hp[:], lhsT=ftTs[kh][:],
                                         rhs=waug[kh][:, 0:272],
                                         start=(kh == 0), stop=(kh == 1))
                    hs = hsbp.tile([128, ROWT], U16, tag="hs")
                    nc.vector.tensor_copy(out=hs[:, 8:256].bitcast(BF16),
                                          in_=hp[:, 8:256])
                    nc.vector.tensor_copy(out=hs[:, 0:4].bitcast(FP8),
                                          in_=hp[:, 0:8])
                    if t == NWIN - 1:
                        # pad ranks (>= NREAL): el = PAD_EL so their exp == 0
                        elm = hsbp.tile([128, NUM_HEADS], F32, tag="elm")
                        nc.vector.tensor_add(
                            out=elm[:], in0=hp[:, 256:264],
                            in1=padel[:].to_broadcast([128, NUM_HEADS]))
                        nc.vector.tensor_copy(out=hs[:, 4:8].bitcast(FP8),
                                              in_=elm[:])
                    else:
                        nc.vector.tensor_copy(out=hs[:, 4:8].bitcast(FP8),
                                              in_=hp[:, 256:264])
                    ers = hsbp.tile([128, NUM_HEADS], F32, tag="ers")
                    nc.vector.tensor_copy(out=ers[:], in_=hp[:, 264:272])
                    nc.sync.dma_start(
                        out=tbl_shard[t * 128:(t + 1) * 128, :],
                        in_=hs[:, :])
                    nc.sync.dma_start(
                        out=er_loc[t * 128:(t + 1) * 128, :], in_=ers[:])

            nc.gpsimd.collective_compute(
                "AllGather", mybir.AluOpType.bypass,
                replica_groups=[list(range(N_CORES))],
                ins=[tbl_shard[:].opt()],
                outs=[tbl_full[:].opt()],
            )

            # ---- aggregation phase ----
            import os as _os
            _phase = _os.environ.get("GAT_PHASE", "full")
            if _phase == "proj":
                with tc.tile_pool(name="dbg", bufs=3) as dbgp:
                    for t in range(NWIN):
                        db = dbgp.tile([128, HF], F32, tag="db")
                        _dbg_src = (tbl_full if _os.environ.get(
                            "GAT_DBG_SHARED") else tbl_shard)
                        nc.sync.dma_start(
                            out=db[:],
                            in_=_dbg_src[t * 128:(t + 1) * 128, 0:HF])
                        nc.sync.dma_start(
                            out=out_d[t * 128:(t + 1) * 128, :], in_=db[:])
            else:
                _agg(nc, tc, constp, sched, ncA, ncB, idx_sb, er_loc,
                     tbl_full, identb, bias_sb, out_d, queues,
                     c_slope, c_eps)
    nc.compile()
    return nc


def _agg(nc, tc, constp, sched, ncA, ncB, idx_sb, er_loc, tbl_full, identb,
         bias_sb, out_d, queues, c_slope, c_eps):
            er_all = constp.tile([128, NWIN, NUM_HEADS], F32)
            nc.sync.dma_start(
                out=er_all[:],
                in_=er_loc[:].rearrange("(w p) h -> p w h", p=128))

            halves = [tbl_full[0:HALF, :], tbl_full[HALF:, :]]
            # group schedule by window
            by_w = [[] for _ in range(NWIN)]
            for gi, (w, h, cols, off) in enumerate(sched):
                by_w[w].append((gi, h, cols, off))
            import os as _os
            nwin_run = int(_os.environ.get("GAT_NWIN", NWIN))

            with (
                tc.tile_pool(name="g", bufs=6) as gp,
                tc.tile_pool(name="m", bufs=4) as mp,
                tc.tile_pool(name="agg", bufs=8, space="PSUM") as aggp,
                tc.tile_pool(name="sm", bufs=3) as smp,
                tc.tile_pool(name="ob", bufs=3) as obp,
            ):
                for w in range(nwin_run):
                    ncw = int(ncA[w] + ncB[w])
                    ps = aggp.tile([128, 264], F32, tag="ps")
                    cg = 0
                    for (gi, h, cols, off) in by_w[w]:
                        gt = gp.tile([128, MAXC, ROWT], U16, tag="g")
                        n = 128 * cols
                        nc.gpsimd.dma_gather(
                            gt[:, 0:cols, :], halves[h],
                            idx_sb[:, off:off + cols * 8],
                            n, n, ROWT, queue_num=queues[gi],
                            single_packet=False)
                        mt = mp.tile([128, MAXC, 264], BF16, tag="m")
                        et = mp.tile([128, MAXC * NUM_HEADS], F32, tag="et")
                        et2 = mp.tile([128, MAXC * NUM_HEADS], F32,
                                      tag="et2")
                        h8f = mp.tile([128, MAXC * NUM_HEADS], F32,
                                      tag="h8")
                        # unpack h[0:8] fp8 (lanes 0-3) + el fp8 (lanes 4-7)
                        etv = et[:, 0:cols * NUM_HEADS].rearrange(
                            "p (c h) -> p c h", h=NUM_HEADS)
                        nc.vector.tensor_copy(
                            out=etv, in_=gt[:, 0:cols, 4:8].bitcast(FP8))
                        h8v = h8f[:, 0:cols * NUM_HEADS].rearrange(
                            "p (c h) -> p c h", h=NUM_HEADS)
                        nc.vector.tensor_copy(
                            out=h8v, in_=gt[:, 0:cols, 0:4].bitcast(FP8))
                        # e = el + er ; leaky ; exp -> p (bf16)
                        erb = er_all[:, w, :][:, None, :].to_broadcast(
                            [128, cols, NUM_HEADS])
                        nc.vector.tensor_add(out=etv, in0=etv, in1=erb)
                        e2v = et2[:, 0:cols * NUM_HEADS]
                        efl = et[:, 0:cols * NUM_HEADS]
                        nc.vector.tensor_mul(
                            out=e2v, in0=efl,
                            in1=c_slope[:].to_broadcast(
                                [128, cols * NUM_HEADS]))
                        nc.vector.tensor_max(out=efl, in0=efl, in1=e2v)
                        pexp = mt[:, 0:cols, 256:264]
                        nc.scalar.activation(
                            out=pexp, in_=etv,
                            func=mybir.ActivationFunctionType.Exp)
                        # weighted messages: head 0 from fp8 lanes + bf16 rest
                        p0 = pexp[:, :, 0:1].to_broadcast([128, cols, 8])
                        nc.vector.tensor_mul(
                            out=mt[:, 0:cols, 0:8], in0=h8v, in1=p0)
                        p0b = pexp[:, :, 0:1].to_broadcast([128, cols, 24])
                        nc.vector.tensor_mul(
                            out=mt[:, 0:cols, 8:32],
                            in0=gt[:, 0:cols, 8:32].bitcast(BF16), in1=p0b)
                        outv = mt[:, 0:cols, 32:256].rearrange(
                            "p c (h f) -> p c h f", h=NUM_HEADS - 1)
                        in0v = gt[:, 0:cols, 32:256].bitcast(BF16).rearrange(
                            "p c (h f) -> p c h f", h=NUM_HEADS - 1)
                        in1v = pexp[:, :, 1:8][:, :, :, None] \
                            .to_broadcast([128, cols, NUM_HEADS - 1,
                                           OUT_FEATS])
                        nc.vector.tensor_mul(out=outv, in0=in0v, in1=in1v)
                        for ci in range(cols):
                            nc.tensor.matmul(
                                ps[:], lhsT=identb[:], rhs=mt[:, ci, :],
                                start=(cg == 0), stop=(cg == ncw - 1))
                            cg += 1
                    fs = obp.tile([128, 264], F32, tag="fs")
                    nc.scalar.copy(out=fs[:], in_=ps[:])
                    sp = smp.tile([128, NUM_HEADS], F32, tag="sp")
                    nc.vector.tensor_add(
                        out=sp[:], in0=fs[:, 256:264],
                        in1=c_eps[:].to_broadcast([128, NUM_HEADS]))
                    rp = smp.tile([128, NUM_HEADS], F32, tag="rp")
                    nc.vector.reciprocal(out=rp[:], in_=sp[:])
                    ob = obp.tile([128, HF], F32, tag="ob")
                    nc.vector.tensor_mul(
                        out=ob[:].rearrange("p (h f) -> p h f", h=NUM_HEADS),
                        in0=fs[:, 0:HF].rearrange("p (h f) -> p h f",
                                                  h=NUM_HEADS),
                        in1=rp[:][:, :, None].to_broadcast(
                            [128, NUM_HEADS, OUT_FEATS]))
                    nc.vector.tensor_add(out=ob[:], in0=ob[:], in1=bias_sb[:])
                    nc.sync.dma_start(
                        out=out_d[w * 128:(w + 1) * 128, :], in_=ob[:])


def kernel(feat, W, attn_l, attn_r, bias, src, dst):
    feat = np.asarray(feat, dtype=np.float32)
    W = np.asarray(W, dtype=np.float32)
    attn_l = np.asarray(attn_l, dtype=np.float32)
    attn_r = np.asarray(attn_r, dtype=np.float32)
    bias = np.asarray(bias, dtype=np.float32)
    src = np.asarray(src).astype(np.int64)
    dst = np.asarray(dst).astype(np.int64)

    perms, sg_all, sched, toti16, idx_imgs, ncA, ncB, queues = _prep(src, dst)
    nc = _build(sched, toti16, ncA, ncB, queues)

    alb = np.tile(attn_l.reshape(1, HF), (128, 1)).astype(np.float32)
    arb = np.tile(attn_r.reshape(1, HF), (128, 1)).astype(np.float32)
    biasb = np.tile(bias.reshape(1, HF), (128, 1)).astype(np.float32)
    ident = np.eye(128, dtype=np.float32)
    import ml_dtypes
    identb = np.eye(128).astype(ml_dtypes.bfloat16)
    padel = np.zeros((128, 1), np.float32)
    padel[NREAL - (NWIN - 1) * 128:] = PAD_EL

    in_maps = []
    for c in range(N_CORES):
        fc = np.zeros((NPAD, IN_FEATS), np.float32)
        fc[:len(perms[c])] = feat[perms[c]]
        in_maps.append({
            "feat": fc, "w": W, "alb": alb, "arb": arb, "biasb": biasb,
            "ident": ident, "identb": identb, "idx": idx_imgs[c],
            "padel": padel,
        })
    res = run_bass_kernel_spmd(nc, in_maps, core_ids=list(range(N_CORES)),
                               trace=False)
    out = np.empty((N_NODES, HF), np.float32)
    for c in range(N_CORES):
        out[perms[c]] = res.results[c]["out"][:len(perms[c])]
    return out.reshape(N_NODES, NUM_HEADS, OUT_FEATS)


if __name__ == "__main__":
    rng = np.random.default_rng(0)
    feat = rng.standard_normal((N_NODES, IN_FEATS), np.float32)
    W = (rng.standard_normal((IN_FEATS, HF), np.float32) * 0.05)
    al = rng.standard_normal((NUM_HEADS, OUT_FEATS), np.float32) * 0.1
    ar = rng.standard_normal((NUM_HEADS, OUT_FEATS), np.float32) * 0.1
    b = np.zeros((NUM_HEADS, OUT_FEATS), np.float32)
    src = rng.integers(0, N_NODES, N_EDGES)
    dst = rng.integers(0, N_NODES, N_EDGES)
    out = kernel(feat=feat, W=W, attn_l=al, attn_r=ar, bias=b,
                 src=src, dst=dst)
    print("out", out.shape, out.dtype, np.abs(out).mean())



# revision 55
# speedup vs baseline: 1.2348x; 1.2348x over previous
"""GAT layer on 8 Trainium2 NeuronCores (Bass/Tile).

Strategy (edge partition by dst, standard 1D graph parallelism):
- Each core owns a contiguous slice of 6250 dst nodes (all edges into them).
- Nodes within a core are re-ordered by in-degree (host-side index prep) so
  that each 128-node PSUM window has near-uniform degree; edge slot (p, c)
  holds the c-th incoming edge of window-node p.  The segment scatter-sum
  then degenerates to matmul-accumulate with a constant identity stationary
  operand, and er[dst] broadcasts per-partition (no per-edge gather).
- Projection h = feat @ [W | Wl | Wr] is node-parallel; the per-node table
  row (768 B: h bf16 + el f32) is AllGathered, then rows are fetched
  per-edge with dma_gather (SWDGE, 4 queues, multi-packet; pad slots rotate
  over 88 pad rows to dodge HBM same-bank serialization).  Softmax runs
  unnormalized (logits are O(1) for this operator) and normalization
  happens per node after aggregation.
- Node->core assignment balances every dst node's in-edges across table
  halves A/B (greedy discrepancy minimization) so the per-window A and B
  slot grids each need only ~deg/2 columns.
"""
import math
import numpy as np
import sys

sys.path.insert(0, "/opt/trn_rl_repo")

from concourse import bass, mybir, bacc, tile
from concourse.bass_utils import run_bass_kernel_spmd

N_NODES = 50000
N_EDGES = 800000
IN_FEATS = 256
NUM_HEADS = 8
OUT_FEATS = 32
HF = NUM_HEADS * OUT_FEATS          # 256
NEG_SLOPE = 0.2
N_CORES = 8
NPC = N_NODES // N_CORES            # 6250 nodes per core
NPAD = 6272                         # 49 * 128
NWIN = NPAD // 128                  # 49
ROWT = 256                          # table row u16 lanes (512 B):
                                    #   lane 0-3:  h[0:8] fp8
                                    #   lane 4-7:  el[0:8] fp8
                                    #   lane 8-255: h[8:256] bf16
TROWS = N_CORES * NPAD              # 50176
HALF = TROWS // 2                   # 25088
PAD_REL = 6271                      # pad row, relative index in either half
NREAL = 6250                        # real nodes per core (rest are pad rows)
PAD_EL = -448.0                     # pad rows' el (fp8 min) -> exp() == 0
MAXC = 8                            # chunk columns per dma_gather
NSQ = 4                             # SWDGE queues
F32 = mybir.dt.float32
BF16 = mybir.dt.bfloat16
U16 = mybir.dt.uint16
I16 = mybir.dt.int16
FP8 = mybir.dt.float8e4


def _degree_rank(degA_slice, degB_slice):
    """Rank nodes by (degA desc, degB desc); with per-dst A/B edge balancing
    degA ~= degB, so windows sorted by degA are near-homogeneous in both
    halves and padding ~= spread of total degree only."""
    order = np.lexsort((-degB_slice, -degA_slice))
    rank = np.empty_like(order)
    rank[order] = np.arange(len(order))
    return order, rank


def _balance_halves(src, dst, deg):
    """Assign nodes to cores so that (a) each degree-octet spreads across all
    8 cores (stratification: per-core degree profiles match) and (b) every
    dst node's in-edges split evenly between src-half A (cores 0-3) and B
    (cores 4-7).  (b) minimizes slot-grid padding: the A and B grids for a
    window each need only ~deg/2 columns."""
    gorder = np.argsort(-deg, kind="stable")
    order_by_src = np.argsort(src, kind="stable")
    dst_sorted = dst[order_by_src]
    starts = np.searchsorted(src[order_by_src], np.arange(N_NODES + 1))
    imb = np.zeros(N_NODES, np.int32)          # degA - degB per dst node
    core_of = np.empty(N_NODES, np.int64)
    for oi in range(0, N_NODES, 8):
        octet = gorder[oi:oi + 8]
        capA = capB = 4
        iA = iB = 0
        for v in octet:
            nb = dst_sorted[starts[v]:starts[v + 1]]
            if capA == 0:
                toA = False
            elif capB == 0:
                toA = True
            elif len(nb) == 0:
                toA = capA >= capB
            else:
                cur = imb[nb]
                cA = np.abs(cur + 1).sum()
                cB = np.abs(cur - 1).sum()
                toA = cA <= cB
            if toA:
                core_of[v] = iA
                iA += 1
                capA -= 1
                if len(nb):
                    imb[nb] += 1
            else:
                core_of[v] = 4 + iB
                iB += 1
                capB -= 1
                if len(nb):
                    imb[nb] -= 1
    return core_of


def _grid_for(p_arr, rel_arr):
    """Edges (partition p, relative table idx) -> grid [128, ncx] int32,
    pad slots = PAD_REL.  Each partition's edges sorted ascending by rel."""
    if len(p_arr) == 0:
        return np.full((128, 1), PAD_REL, np.int32)
    o = np.lexsort((rel_arr, p_arr))
    p_arr, rel_arr = p_arr[o], rel_arr[o]
    counts = np.bincount(p_arr, minlength=128)
    ncx = int(counts.max())
    grid = np.full((128, ncx), PAD_REL, np.int32)
    # column index within each partition = running position
    starts = np.zeros(128, np.int64)
    starts[1:] = np.cumsum(counts)[:-1]
    col = np.arange(len(p_arr)) - starts[p_arr]
    grid[p_arr, col] = rel_arr
    return grid


def _pack_gather(grid_cols):
    """grid_cols [128, cols] -> wrapped idx image [128, n/16] int16."""
    cols = grid_cols.shape[1]
    n = 128 * cols
    flat = np.empty(n, np.int32)
    P = np.arange(128)[:, None]
    CI = np.arange(cols)[None, :]
    pos = (P % 16) * (n // 16) + (P // 16) + 8 * CI
    flat[pos.ravel()] = grid_cols.ravel()
    return np.tile(flat.reshape(16, n // 16), (8, 1)).astype(np.int16)


def _prep(src, dst):
    """All host-side index preprocessing.  Returns per-core arrays + the
    (shared) gather schedule."""
    # Stratified node->core assignment with per-dst A/B edge balancing.
    deg = np.bincount(dst, minlength=N_NODES)
    core_of = _balance_halves(src, dst, deg)
    srcA = core_of[src] < (N_CORES // 2)        # src in table half A
    degA = np.bincount(dst[srcA], minlength=N_NODES)
    degB = np.bincount(dst[~srcA], minlength=N_NODES)
    grank = np.empty(N_NODES, np.int64)
    perms = []
    for c in range(N_CORES):
        ids = np.nonzero(core_of == c)[0]       # global node ids of core c
        order, rank = _degree_rank(degA[ids], degB[ids])
        perms.append(ids[order])                # rank -> global node id
        grank[ids] = c * NPAD + rank
    sg_all = grank[src]
    dcore = core_of[dst]
    dr_all = grank[dst] - dcore * NPAD          # local rank of dst

    # per (core, window, half) grids
    grids = [[[None, None] for _ in range(NWIN)] for _ in range(N_CORES)]
    ncx = np.zeros((N_CORES, NWIN, 2), np.int64)
    for c in range(N_CORES):
        sel = np.nonzero(dcore == c)[0]
        sg = sg_all[sel]
        dr = dr_all[sel]
        w_arr = dr // 128
        p_arr = dr % 128
        half = (sg >= HALF).astype(np.int64)
        rel = np.where(half == 0, sg, sg - HALF)
        key = w_arr * 2 + half
        order = np.argsort(key, kind="stable")
        ksort = key[order]
        bounds = np.searchsorted(ksort, np.arange(NWIN * 2 + 1))
        for w in range(NWIN):
            for h in range(2):
                lo, hi = bounds[w * 2 + h], bounds[w * 2 + h + 1]
                idxs = order[lo:hi]
                g = _grid_for(p_arr[idxs].astype(np.int64),
                              rel[idxs].astype(np.int64))
                grids[c][w][h] = g
                ncx[c, w, h] = g.shape[1]

    # shared schedule: per (window, half) chunk count = max over cores
    ncA = ncx[:, :, 0].max(axis=0)
    ncB = ncx[:, :, 1].max(axis=0)
    ncA = np.maximum(ncA, 1)                    # ensure >=1 chunk per window
    sched = []                                  # (w, half, cols, off16)
    off16 = 0
    for w in range(NWIN):
        for h, nc_w in ((0, int(ncA[w])), (1, int(ncB[w]))):
            for j0 in range(0, nc_w, MAXC):
                cols = min(MAXC, nc_w - j0)
                if cols <= 0:
                    continue
                sched.append((w, h, cols, off16))
                off16 += cols * 8               # n/16 = 128*cols/16
    toti16 = off16
    # queue = emission index % NSQ: Tile assigns SWDGE DMAs to the 8 DMASW
    # sem lanes round-robin in emission order, and each lane must stay
    # locked to a single SWDGE queue (8 % NSQ == 0 keeps them aligned).
    queues = [i % NSQ for i in range(len(sched))]

    # per-core idx images following the shared schedule.  Pad slots rotate
    # over all 88 pad rows of a half (4 cores x ranks 6250-6271) instead of
    # hammering one row: concurrent same-address HBM reads serialize on a
    # single bank.
    padpool = np.array([c * NPAD + r for c in range(4)
                        for r in range(NREAL, NPAD)], np.int32)
    idx_imgs = []
    for c in range(N_CORES):
        img = np.empty((128, toti16), np.int16)
        phase = 0
        for (w, h, cols, off) in sched:
            g = grids[c][w][h]
            gc = np.full((128, cols), PAD_REL, np.int32)
            # j0 for this gather: count previous gathers of same (w, h)
            prev = sum(cc for (w2, h2, cc, o2) in sched
                       if w2 == w and h2 == h and o2 < off)
            j0 = prev
            avail = max(0, min(cols, g.shape[1] - j0))
            if avail > 0:
                gc[:, :avail] = g[:, j0:j0 + avail]
            mask = gc == PAD_REL
            npads = int(mask.sum())
            if npads:
                gc[mask] = padpool[(np.arange(npads) + phase)
                                   % len(padpool)]
                phase += npads
            img[:, off:off + cols * 8] = _pack_gather(gc)
        idx_imgs.append(img)
    return perms, sg_all, sched, toti16, idx_imgs, ncA, ncB, queues


def _build(sched, toti16, ncA, ncB, queues):
    nc = bacc.Bacc("TRN2", target_bir_lowering=False, debug=False,
                   num_devices=N_CORES, num_swdge_queues=NSQ)
    featw_in = nc.dram_tensor("featw", [128, 2 * NWIN * 128], BF16,
                              kind="ExternalInput")
    w_in = nc.dram_tensor("w", [IN_FEATS, HF], F32, kind="ExternalInput")
    alb_in = nc.dram_tensor("alb", [128, HF], F32, kind="ExternalInput")
    arb_in = nc.dram_tensor("arb", [128, HF], F32, kind="ExternalInput")
    bias_in = nc.dram_tensor("biasb", [128, HF], F32, kind="ExternalInput")
    idb_in = nc.dram_tensor("identb", [128, 128], BF16, kind="ExternalInput")
    idx_in = nc.dram_tensor("idx", [128, toti16], I16, kind="ExternalInput")
    padel_in = nc.dram_tensor("padel", [128, 1], F32, kind="ExternalInput")
    out_d = nc.dram_tensor("out", [NPAD, HF], F32, kind="ExternalOutput")

    with tile.TileContext(nc) as tc:
        with (
            tc.tile_pool(name="const", bufs=1) as constp,
            tc.tile_pool(name="dram", bufs=1, space="DRAM") as dramp,
        ):
            tbl_shard = dramp.tile([NPAD, ROWT], U16)
            er_loc = dramp.tile([NPAD, NUM_HEADS], F32)
            tbl_full = dramp.tile([TROWS, ROWT], U16, addr_space="Shared")

            identb = constp.tile([128, 128], BF16)
            nc.sync.dma_start(out=identb[:], in_=idb_in[:, :])
            bias_sb = constp.tile([128, HF], F32)
            nc.sync.dma_start(out=bias_sb[:], in_=bias_in[:, :])
            idx_sb = constp.tile([128, toti16], I16)
            nc.sync.dma_start(out=idx_sb[:], in_=idx_in[:, :])
            alb = constp.tile([128, HF], F32)
            nc.sync.dma_start(out=alb[:], in_=alb_in[:, :])
            arb = constp.tile([128, HF], F32)
            nc.sync.dma_start(out=arb[:], in_=arb_in[:, :])
            padel = constp.tile([128, 1], F32)
            nc.sync.dma_start(out=padel[:], in_=padel_in[:, :])
            c_slope = constp.tile([128, 1], F32)
            nc.vector.memset(c_slope[:], NEG_SLOPE)
            c_eps = constp.tile([128, 1], F32)
            nc.vector.memset(c_eps[:], 1e-30)

            # ---- W_aug: [W | Wl | Wr] per k-half ----
            waug = []
            with tc.tile_pool(name="wtmp", bufs=2) as wtmp:
                for kh in range(2):
                    wa = constp.tile([128, 272], F32, name=f"waug{kh}",
                                     tag=f"waug{kh}")
                    nc.sync.dma_start(out=wa[:, 0:HF],
                                      in_=w_in[kh * 128:(kh + 1) * 128, :])
                    tmp = wtmp.tile([128, HF], F32, tag="wt")
                    nc.vector.tensor_mul(out=tmp[:], in0=wa[:, 0:HF],
                                         in1=alb[:])
                    nc.vector.tensor_reduce(
                        out=wa[:, 256:264],
                        in_=tmp[:].rearrange("p (h f) -> p h f", h=NUM_HEADS),
                        axis=mybir.AxisListType.X, op=mybir.AluOpType.add)
                    tmp2 = wtmp.tile([128, HF], F32, tag="wt2")
                    nc.vector.tensor_mul(out=tmp2[:], in0=wa[:, 0:HF],
                                         in1=arb[:])
                    nc.vector.tensor_reduce(
                        out=wa[:, 264:272],
                        in_=tmp2[:].rearrange("p (h f) -> p h f", h=NUM_HEADS),
                        axis=mybir.AxisListType.X, op=mybir.AluOpType.add)
                    waug.append(wa)

            # bf16 copies of W_aug for bf16 projection matmuls
            waugb = []
            for kh in range(2):
                wb = constp.tile([128, 272], BF16, name=f"waugb{kh}",
                                 tag=f"waugb{kh}")
                nc.vector.tensor_copy(out=wb[:], in_=waug[kh][:])
                waugb.append(wb)
            # whole pre-transposed bf16 feature set in one contiguous DMA;
            # lhsT of window t / k-half kh is an SBUF view (no transposes)
            ftall = constp.tile([128, 2 * NWIN * 128], BF16)
            nc.sync.dma_start(out=ftall[:], in_=featw_in[:, :])
            ftv = ftall[:].rearrange("p (kh t n) -> p kh t n", kh=2, t=NWIN)

            # ---- projection phase ----
            with (
                tc.tile_pool(name="hps", bufs=3, space="PSUM") as hpsp,
                tc.tile_pool(name="hsb", bufs=6) as hsbp,
            ):
                for t in range(NWIN):
                    hp = hpsp.tile([128, 272], F32, tag="hp")
                    for kh in range(2):
                        nc.tensor.matmul(hp[:], lhsT=ftv[:, kh, t, :],
                                         rhs=waugb[kh][:, 0:272],
                                         start=(kh == 0), stop=(kh == 1))
                    hs = hsbp.tile([128, ROWT], U16, tag="hs")
                    nc.vector.tensor_copy(out=hs[:, 8:256].bitcast(BF16),
                                          in_=hp[:, 8:256])
                    nc.vector.tensor_copy(out=hs[:, 0:4].bitcast(FP8),
                                          in_=hp[:, 0:8])
                    if t == NWIN - 1:
                        # pad ranks (>= NREAL): el = PAD_EL so their exp == 0
                        elm = hsbp.tile([128, NUM_HEADS], F32, tag="elm")
                        nc.vector.tensor_add(
                            out=elm[:], in0=hp[:, 256:264],
                            in1=padel[:].to_broadcast([128, NUM_HEADS]))
                        nc.vector.tensor_copy(out=hs[:, 4:8].bitcast(FP8),
                                              in_=elm[:])
                    else:
                        nc.vector.tensor_copy(out=hs[:, 4:8].bitcast(FP8),
                                              in_=hp[:, 256:264])
                    ers = hsbp.tile([128, NUM_HEADS], F32, tag="ers")
                    nc.vector.tensor_copy(out=ers[:], in_=hp[:, 264:272])
                    nc.sync.dma_start(
                        out=tbl_shard[t * 128:(t + 1) * 128, :],
                        in_=hs[:, :])
                    nc.sync.dma_start(
                        out=er_loc[t * 128:(t + 1) * 128, :], in_=ers[:])

            nc.gpsimd.collective_compute(
                "AllGather", mybir.AluOpType.bypass,
                replica_groups=[list(range(N_CORES))],
                ins=[tbl_shard[:].opt()],
                outs=[tbl_full[:].opt()],
            )

            # ---- aggregation phase ----
            import os as _os
            _phase = _os.environ.get("GAT_PHASE", "full")
            if _phase == "proj":
                with tc.tile_pool(name="dbg", bufs=3) as dbgp:
                    for t in range(NWIN):
                        db = dbgp.tile([128, HF], F32, tag="db")
                        _dbg_src = (tbl_full if _os.environ.get(
                            "GAT_DBG_SHARED") else tbl_shard)
                        nc.sync.dma_start(
                            out=db[:],
                            in_=_dbg_src[t * 128:(t + 1) * 128, 0:HF])
                        nc.sync.dma_start(
                            out=out_d[t * 128:(t + 1) * 128, :], in_=db[:])
            else:
                _agg(nc, tc, constp, sched, ncA, ncB, idx_sb, er_loc,
                     tbl_full, identb, bias_sb, out_d, queues,
                     c_slope, c_eps)
    nc.compile()
    return nc


def _agg(nc, tc, constp, sched, ncA, ncB, idx_sb, er_loc, tbl_full, identb,
         bias_sb, out_d, queues, c_slope, c_eps):
            er_all = constp.tile([128, NWIN, NUM_HEADS], F32)
            nc.sync.dma_start(
                out=er_all[:],
                in_=er_loc[:].rearrange("(w p) h -> p w h", p=128))

            halves = [tbl_full[0:HALF, :], tbl_full[HALF:, :]]
            # group schedule by window
            by_w = [[] for _ in range(NWIN)]
            for gi, (w, h, cols, off) in enumerate(sched):
                by_w[w].append((gi, h, cols, off))
            import os as _os
            nwin_run = int(_os.environ.get("GAT_NWIN", NWIN))

            with (
                tc.tile_pool(name="g", bufs=12) as gp,
                tc.tile_pool(name="m", bufs=8) as mp,
                tc.tile_pool(name="agg", bufs=8, space="PSUM") as aggp,
                tc.tile_pool(name="sm", bufs=3) as smp,
                tc.tile_pool(name="ob", bufs=3) as obp,
            ):
                for w in range(nwin_run):
                    ncw = int(ncA[w] + ncB[w])
                    ps = aggp.tile([128, 264], F32, tag="ps")
                    cg = 0
                    for (gi, h, cols, off) in by_w[w]:
                        gt = gp.tile([128, MAXC, ROWT], U16, tag="g")
                        n = 128 * cols
                        nc.gpsimd.dma_gather(
                            gt[:, 0:cols, :], halves[h],
                            idx_sb[:, off:off + cols * 8],
                            n, n, ROWT, queue_num=queues[gi],
                            single_packet=False)
                        mt = mp.tile([128, MAXC, 264], BF16, tag="m")
                        et = mp.tile([128, MAXC * NUM_HEADS], F32, tag="et")
                        et2 = mp.tile([128, MAXC * NUM_HEADS], F32,
                                      tag="et2")
                        h8f = mp.tile([128, MAXC * NUM_HEADS], F32,
                                      tag="h8")
                        # unpack h[0:8] fp8 (lanes 0-3) + el fp8 (lanes 4-7)
                        etv = et[:, 0:cols * NUM_HEADS].rearrange(
                            "p (c h) -> p c h", h=NUM_HEADS)
                        nc.vector.tensor_copy(
                            out=etv, in_=gt[:, 0:cols, 4:8].bitcast(FP8))
                        h8v = h8f[:, 0:cols * NUM_HEADS].rearrange(
                            "p (c h) -> p c h", h=NUM_HEADS)
                        nc.vector.tensor_copy(
                            out=h8v, in_=gt[:, 0:cols, 0:4].bitcast(FP8))
                        # e = el + er ; leaky ; exp -> p (bf16)
                        erb = er_all[:, w, :][:, None, :].to_broadcast(
                            [128, cols, NUM_HEADS])
                        nc.vector.tensor_add(out=etv, in0=etv, in1=erb)
                        e2v = et2[:, 0:cols * NUM_HEADS]
                        efl = et[:, 0:cols * NUM_HEADS]
                        nc.vector.tensor_mul(
                            out=e2v, in0=efl,
                            in1=c_slope[:].to_broadcast(
                                [128, cols * NUM_HEADS]))
                        nc.vector.tensor_max(out=efl, in0=efl, in1=e2v)
                        pexp = mt[:, 0:cols, 256:264]
                        nc.scalar.activation(
                            out=pexp, in_=etv,
                            func=mybir.ActivationFunctionType.Exp)
                        # weighted messages: head 0 from fp8 lanes + bf16 rest
                        p0 = pexp[:, :, 0:1].to_broadcast([128, cols, 8])
                        nc.vector.tensor_mul(
                            out=mt[:, 0:cols, 0:8], in0=h8v, in1=p0)
                        p0b = pexp[:, :, 0:1].to_broadcast([128, cols, 24])
                        nc.vector.tensor_mul(
                            out=mt[:, 0:cols, 8:32],
                            in0=gt[:, 0:cols, 8:32].bitcast(BF16), in1=p0b)
                        outv = mt[:, 0:cols, 32:256].rearrange(
                            "p c (h f) -> p c h f", h=NUM_HEADS - 1)
                        in0v = gt[:, 0:cols, 32:256].bitcast(BF16).rearrange(
                            "p c (h f) -> p c h f", h=NUM_HEADS - 1)
                        in1v = pexp[:, :, 1:8][:, :, :, None] \
                            .to_broadcast([128, cols, NUM_HEADS - 1,
                                           OUT_FEATS])
                        nc.vector.tensor_mul(out=outv, in0=in0v, in1=in1v)
                        for ci in range(cols):
                            nc.tensor.matmul(
                                ps[:], lhsT=identb[:], rhs=mt[:, ci, :],
                                start=(cg == 0), stop=(cg == ncw - 1))
                            cg += 1
                    fs = obp.tile([128, 264], F32, tag="fs")
                    nc.scalar.copy(out=fs[:], in_=ps[:])
                    sp = smp.tile([128, NUM_HEADS], F32, tag="sp")
                    nc.vector.tensor_add(
                        out=sp[:], in0=fs[:, 256:264],
                        in1=c_eps[:].to_broadcast([128, NUM_HEADS]))
                    rp = smp.tile([128, NUM_HEADS], F32, tag="rp")
                    nc.vector.reciprocal(out=rp[:], in_=sp[:])
                    ob = obp.tile([128, HF], F32, tag="ob")
                    nc.vector.tensor_mul(
                        out=ob[:].rearrange("p (h f) -> p h f", h=NUM_HEADS),
                        in0=fs[:, 0:HF].rearrange("p (h f) -> p h f",
                                                  h=NUM_HEADS),
                        in1=rp[:][:, :, None].to_broadcast(
                            [128, NUM_HEADS, OUT_FEATS]))
                    nc.vector.tensor_add(out=ob[:], in0=ob[:], in1=bias_sb[:])
                    nc.sync.dma_start(
                        out=out_d[w * 128:(w + 1) * 128, :], in_=ob[:])


def kernel(feat, W, attn_l, attn_r, bias, src, dst):
    feat = np.asarray(feat, dtype=np.float32)
    W = np.asarray(W, dtype=np.float32)
    attn_l = np.asarray(attn_l, dtype=np.float32)
    attn_r = np.asarray(attn_r, dtype=np.float32)
    bias = np.asarray(bias, dtype=np.float32)
    src = np.asarray(src).astype(np.int64)
    dst = np.asarray(dst).astype(np.int64)

    perms, sg_all, sched, toti16, idx_imgs, ncA, ncB, queues = _prep(src, dst)
    nc = _build(sched, toti16, ncA, ncB, queues)

    alb = np.tile(attn_l.reshape(1, HF), (128, 1)).astype(np.float32)
    arb = np.tile(attn_r.reshape(1, HF), (128, 1)).astype(np.float32)
    biasb = np.tile(bias.reshape(1, HF), (128, 1)).astype(np.float32)
    import ml_dtypes
    identb = np.eye(128).astype(ml_dtypes.bfloat16)
    padel = np.zeros((128, 1), np.float32)
    padel[NREAL - (NWIN - 1) * 128:] = PAD_EL

    in_maps = []
    for c in range(N_CORES):
        fc = np.zeros((NPAD, IN_FEATS), np.float32)
        fc[:len(perms[c])] = feat[perms[c]]
        # [feat%128, kh, window, node] so lhsT tiles are SBUF views
        fw = fc.reshape(NWIN, 128, 2, 128).transpose(3, 2, 0, 1) \
               .reshape(128, 2 * NWIN * 128).astype(ml_dtypes.bfloat16)
        in_maps.append({
            "featw": np.ascontiguousarray(fw), "w": W, "alb": alb,
            "arb": arb, "biasb": biasb,
            "identb": identb, "idx": idx_imgs[c],
            "padel": padel,
        })
    res = run_bass_kernel_spmd(nc, in_maps, core_ids=list(range(N_CORES)),
                               trace=False)
    out = np.empty((N_NODES, HF), np.float32)
    for c in range(N_CORES):
        out[perms[c]] = res.results[c]["out"][:len(perms[c])]
    return out.reshape(N_NODES, NUM_HEADS, OUT_FEATS)


if __name__ == "__main__":
    rng = np.random.default_rng(0)
    feat = rng.standard_normal((N_NODES, IN_FEATS), np.float32)
    W = (rng.standard_normal((IN_FEATS, HF), np.float32) * 0.05)
    al = rng.standard_normal((NUM_HEADS, OUT_FEATS), np.float32) * 0.1
    ar = rng.standard_normal((NUM_HEADS, OUT_FEATS), np.float32) * 0.1
    b = np.zeros((NUM_HEADS, OUT_FEATS), np.float32)
    src = rng.integers(0, N_NODES, N_EDGES)
    dst = rng.integers(0, N_NODES, N_EDGES)
    out = kernel(feat=feat, W=W, attn_l=al, attn_r=ar, bias=b,
                 src=src, dst=dst)
    print("out", out.shape, out.dtype, np.abs(out).mean())

